# revision 38
# baseline (speedup 1.0000x reference)
"""BEiT-style attention (B=64, N=197, C=768, H=12, rel-pos bias) on 8 TRN2 cores.

Data-parallel over batch: 8 batch items per core, no collectives.

Design notes (what made this fast — 232us baseline -> ~166us):
  - Engines execute their queues IN ORDER, so emission order IS the
    schedule, and the TRN2 PE has p-states (full 2.4 GHz only after ~3us
    of continuous execution; a stall drops it to 1.2 GHz). The whole
    kernel is one software-pipelined stream in which the tensor engine
    never head-blocks: per head-pair p, S/exp runs two b-iterations
    ahead of O/denom, and the qk GEMM of pair p+1 (proj chunks during
    p=5) is emitted block-wise between them as tensor filler that covers
    the exp->mul->O latency.
  - Rel-pos bias enters as a multiplicative exp(bias) table applied to
    exp(S) on the DVE (2-byte 2x mode) instead of an identity-matmul
    PSUM prefill; softmax denominators use ones-matmuls batched over
    b-pairs, and the ~5x-faster reciprocal_approx_fast custom DVE op.
  - All attention matmuls use K=128 (token chunks padded cross-batch;
    the exp(bias) table zeroes the pad rows so they contribute nothing).
  - Projection runs over 13 flat 128-token chunks (MPAD = 1664 = 13*128);
    proj bias is added by the DVE during PSUM evacuation (no ones-row
    matmul). v_bias is folded into the proj bias on the host (softmax
    rows sum to 1); q_bias/scale fold into the qk weights / ACT evac.
  - Every DRAM input is host-prearranged to its exact SBUF layout; the
    wv/x loads are split so the first v matmul starts ~5us earlier, and
    ~22 warmup matmuls on dummy data keep the PE busy (and its p-state
    ramped) during the input-DMA wait — the v stage then starts at full
    clock with all inputs resident.
    (Empirically, finer DMA splitting [3+ pieces of a hot tile] or
    fine-grained interleaving of accumulation groups makes ALL matmuls
    ~20% slower — keep hot tiles to <=2 DMA pieces and keep matmul
    accumulation groups contiguous.)
"""

import numpy as np
import ml_dtypes

import concourse.bass as bass
import concourse.mybir as mybir
import concourse.tile as tile
from concourse import bacc
from concourse.bass_utils import run_bass_kernel_spmd

BF16 = ml_dtypes.bfloat16
F32 = mybir.dt.float32
BF = mybir.dt.bfloat16

B, N, C = 64, 197, 768
H, HD = 12, 64
NCORES = 8
BL = B // NCORES            # 8 batches per core
M = BL * N                  # 1576 real tokens per core
MPAD = 1664                 # 13 * 128
SCALE = HD ** -0.5
MCH = [(0, 512), (512, 512), (1024, 512), (1536, 128)]
FCH = [(0, 512), (512, 256)]

_NC = None


def _build():
    nc = bacc.Bacc("TRN2", target_bir_lowering=False, debug=False)

    x = nc.dram_tensor("x", [128, 6, MPAD], BF, kind="ExternalInput")
    wqk = nc.dram_tensor("wqk", [128, 12 * C], BF, kind="ExternalInput")
    wv = nc.dram_tensor("wv", [128, 6, C], BF, kind="ExternalInput")
    wp = nc.dram_tensor("wp", [128, 6 * C], BF, kind="ExternalInput")
    eb = nc.dram_tensor("eb", [128, 12 * 512], BF, kind="ExternalInput")
    qb = nc.dram_tensor("qb", [128, 6], F32, kind="ExternalInput")
    pbb = nc.dram_tensor("pbb", [128, C], BF, kind="ExternalInput")
    out = nc.dram_tensor("out", [M, C], BF, kind="ExternalOutput")

    Ident = mybir.ActivationFunctionType.Identity
    Copy = mybir.ActivationFunctionType.Copy
    Exp = mybir.ActivationFunctionType.Exp

    with tile.TileContext(nc) as tc:
        with (
            tc.tile_pool(name="persist", bufs=1) as P,
            tc.tile_pool(name="et", bufs=5) as et_pool,
            tc.tile_pool(name="rcp", bufs=4) as r_pool,
            tc.tile_pool(name="ob", bufs=3) as ob_pool,
            tc.tile_pool(name="mm", bufs=8, space="PSUM") as mm,
        ):
            # ---- inputs to SBUF (one DMA per tensor, pre-laid-out) ----
            # wv/x split by first use; more than 2 pieces per hot tile
            # measurably slows every matmul down (see module docstring).
            wv_sb = P.tile([128, 6, C], BF, tag="wv")
            xT = P.tile([128, 6, MPAD], BF, tag="xt")
            # staged by first use: the first v matmuls (batch 0, f-half 0)
            # need only wv cols 0:512 and x tokens 0:256; the 640 split
            # point buys ~9.6us of v compute before the x tail is needed
            nc.sync.dma_start(wv_sb[:, :, 0:512], wv[:, :, 0:512])
            nc.sync.dma_start(xT[:, :, 0:640], x[:, :, 0:640])
            nc.sync.dma_start(wv_sb[:, :, 512:C], wv[:, :, 512:C])
            nc.sync.dma_start(xT[:, :, 640:MPAD], x[:, :, 640:MPAD])
            wqk_sb = P.tile([128, 12, C], BF, tag="wqk")
            nc.sync.dma_start(wqk_sb[:, :, :], wqk[:, :])
            eb_sb = P.tile([128, 12, 512], BF, tag="eb")
            nc.sync.dma_start(eb_sb[:, :, :], eb[:, :])
            wp_sb = P.tile([128, 6, C], BF, tag="wp")
            nc.sync.dma_start(wp_sb[:, :, :], wp[:, :])
            qb_sb = P.tile([128, 6], F32, tag="qb")
            nc.sync.dma_start(qb_sb[:, :], qb[:, :])
            pbb_sb = P.tile([128, C], BF, tag="pbb")
            nc.sync.dma_start(pbb_sb[:, :], pbb[:, :])

            ones64 = P.tile([128, 64], BF, tag="ones64")
            nc.gpsimd.memset(ones64[:, :], 1.0)

            # Warmup: the PE p-state ramps to full clock only after ~3us
            # of continuous execution, and the first real matmul waits
            # ~15us for input DMA. Keep the PE busy on dummy data during
            # the wait so the v stage starts at full clock (~3us saved;
            # the dummies themselves finish right as the inputs land).
            dummy = P.tile([128, 512], BF, tag="dummy")
            nc.gpsimd.memset(dummy[:, :], 0.0)
            for w in range(23):
                wps = mm.tile([128, 512], F32, tag="mm", name="warm")
                nc.tensor.matmul(
                    wps[:64, :],
                    lhsT=ones64[:, :],
                    rhs=dummy[:, :],
                    start=True,
                    stop=True,
                )

            qkT = P.tile([128, 12, MPAD], BF, tag="qkt")
            v_sb = P.tile([128, BL, 2, C], BF, tag="v")
            AT = P.tile([128, 6, MPAD], BF, tag="at")
            # zero the pad-token tail so proj chunk 12 reads finite values
            nc.gpsimd.memset(AT[:, :, M:MPAD], 0.0)

            # ---- stage V: v[m, d] per-batch 128-token chunks ----
            # ch1 rows 69:128 hold cross-batch/pad tokens; their E rows are
            # zeroed by the exp(bias) table so they never contribute.
            for b in range(BL):
                for ch in range(2):
                    t0 = b * N + 128 * ch
                    for f0, fw in FCH:
                        ps = mm.tile([128, 512], F32, tag="mm")
                        for ct in range(6):
                            nc.tensor.matmul(
                                ps[:, :fw],
                                lhsT=xT[:, ct, t0 : t0 + 128],
                                rhs=wv_sb[:, ct, f0 : f0 + fw],
                                start=(ct == 0),
                                stop=(ct == 5),
                            )
                        nc.scalar.activation(
                            v_sb[:, b, ch, f0 : f0 + fw], ps[:, :fw], Copy
                        )

            # ---- p-loop with software pipelining ----
            # Engines execute their queues IN ORDER, so the emission order
            # IS the schedule. Per pair p, S/exp/mul runs two b-iterations
            # ahead of O/denom, and the qk GEMM groups of pair p+1 (proj
            # chunks during p=5) are interleaved between them as tensor
            # filler — the tensor queue never head-blocks on the
            # exp->mul->O dependency chain and the PE stays at full
            # p-state.

            def emit_qk_group(r, p_of_r, chunks):
                for a in qk_group_actions(r, p_of_r, chunks):
                    a()

            def qk_group_actions(r, p_of_r, chunks):
                # two m-chunks per group, ct-inner with both banks held:
                # consecutive matmuls share the same stationary weights.
                # Returned as single-matmul thunks so the b-loop can pump
                # them between the short attention matmuls — their long
                # streams hide the attention stationary-load time.
                pss = []

                def mk(ct, ci, m0, mw):
                    def t():
                        if ct == 0 and ci == 0:
                            pss[:] = [
                                mm.tile([128, 512], F32, tag="mm", name=f"qkps{i}")
                                for i in range(len(chunks))
                            ]
                        nc.tensor.matmul(
                            pss[ci][:, :mw],
                            lhsT=wqk_sb[:, r, 128 * ct : 128 * (ct + 1)],
                            rhs=xT[:, ct, m0 : m0 + mw],
                            start=(ct == 0),
                            stop=(ct == 5),
                        )
                    return t

                def ev():
                    for ps, (m0, mw) in zip(pss, chunks):
                        if r == p_of_r:  # q rows: scaled bias during evac
                            nc.scalar.activation(
                                qkT[:, r, m0 : m0 + mw],
                                ps[:, :mw],
                                Ident,
                                bias=qb_sb[:, p_of_r : p_of_r + 1],
                            )
                        else:  # k rows: plain cast on the DVE
                            nc.vector.tensor_copy(
                                qkT[:, r, m0 : m0 + mw], ps[:, :mw]
                            )

                acts = [
                    mk(ct, ci, m0, mw)
                    for ct in range(6)
                    for ci, (m0, mw) in enumerate(chunks)
                ]
                acts.append(ev)
                return acts

            def qk_groups(p):
                # q tail chunk stops at M (S reads q only to 1576); k tail
                # extends to 1635 (ch1 cross-batch lhsT columns)
                for lo in (True, False):
                    for r in (p, 6 + p):
                        if lo:
                            yield (r, p, ((0, 512), (512, 512)))
                        else:
                            yield (r, p, ((1024, 512), (1536, 40 if r == p else 99)))

            def emit_S(p, b, et2):
                # S^T[m, n] per head in its own bank; the odd head's lhsT
                # sits at partition base 64 (disjoint PE row groups run
                # concurrently; drains go to different banks). Then
                # exp(S^T) -> E0 and E = E0 * exp(bias) on the DVE
                # (2-byte 2x mode); the table zeroes junk cols 197:256
                # and the ch1 pad rows 69:128. Results land in half b%2
                # of the pair tile et2.
                bn = b * N
                bi = b % 2
                psH = [
                    mm.tile([128, 512], F32, tag="mm", name=f"psH{hj}")
                    for hj in range(2)
                ]
                for hj in range(2):
                    hs = slice(64 * hj, 64 * (hj + 1))
                    for ch in range(2):
                        nc.tensor.matmul(
                            psH[hj][:, 256 * ch : 256 * ch + N],
                            lhsT=qkT[hs, 6 + p, bn + 128 * ch : bn + 128 * ch + 128],
                            rhs=qkT[hs, p, bn : bn + N],
                            start=True,
                            stop=True,
                        )
                for hj in range(2):
                    nc.scalar.activation(
                        et2[:, bi, hj, 0:453], psH[hj][:, 0:453], Exp
                    )
                nc.vector.tensor_mul(
                    et2[:, bi, :, 0:453],
                    et2[:, bi, :, 0:453],
                    eb_sb[:, 2 * p : 2 * p + 2, 0:453],
                )

            def emit_O(p, b, et2, psODp):
                # O^T for batch b into column half b%2 of the pair bank
                bi = b % 2
                for hj in range(2):
                    od = slice(64 * hj, 64 * (hj + 1))
                    for ch in range(2):
                        nc.tensor.matmul(
                            psODp[od, 256 * bi : 256 * bi + N],
                            lhsT=v_sb[:, b, ch, (2 * p + hj) * HD : (2 * p + hj + 1) * HD],
                            rhs=et2[:, bi, hj, 256 * ch : 256 * ch + N],
                            start=(ch == 0),
                            stop=(ch == 1),
                        )

            def emit_D(p, k, et2, psODp):
                # denominators for the whole b-pair: per head, one
                # accumulation over both token chunks with the pair's two
                # E tiles side by side in the free dim (2*197 cols) —
                # half the matmuls and stationary loads of per-b denoms
                psD = mm.tile([128, 512], F32, tag="mm", name="psD")
                for hj in range(2):
                    od = slice(64 * hj, 64 * (hj + 1))
                    for ch in range(2):
                        nc.tensor.matmul(
                            psD[od, 0 : 2 * N],
                            lhsT=ones64[:, :],
                            rhs=et2[:, 0:2, hj, 256 * ch : 256 * ch + N],
                            start=(ch == 0),
                            stop=(ch == 1),
                        )
                rcp = r_pool.tile([128, 2 * N], F32, tag="rcp")
                nc.vector.reciprocal_approx_fast(
                    out=rcp[:, :], in_=psD[:, 0 : 2 * N]
                )
                for bi in range(2):
                    bn = (2 * k + bi) * N
                    nc.vector.tensor_mul(
                        AT[:, p, bn : bn + N],
                        psODp[:, 256 * bi : 256 * bi + N],
                        rcp[:, N * bi : N * bi + N],
                    )

            def emit_proj(k):
                m0 = 128 * k
                nv = min(128, M - m0)  # valid rows (last chunk: 40)
                ob = ob_pool.tile([128, C], BF, tag="ob")
                for f0, fw in FCH:
                    ps = mm.tile([128, 512], F32, tag="mm")
                    for ct in range(6):
                        nc.tensor.matmul(
                            ps[:, :fw],
                            lhsT=AT[:, ct, m0 : m0 + 128],
                            rhs=wp_sb[:, ct, f0 : f0 + fw],
                            start=(ct == 0),
                            stop=(ct == 5),
                        )
                    nc.vector.tensor_add(
                        ob[:nv, f0 : f0 + fw],
                        ps[:nv, :fw],
                        pbb_sb[:nv, f0 : f0 + fw],
                    )
                nc.sync.dma_start(out[m0 : m0 + nv, :], ob[:nv, :])

            # proj chunk k is ready once attn(p=5, b) is done for all
            # batches its 128-token window touches
            proj_after_b = [[] for _ in range(BL)]
            for k in range(13):
                proj_after_b[min((128 * k + 127) // N, BL - 1)].append(k)

            for g in qk_groups(0):  # prologue: pair 0's projections
                emit_qk_group(*g)
            seq = [(p, b) for p in range(6) for b in range(BL)]
            gsrc = {p: iter(qk_groups(p + 1)) for p in range(5)}

            ets = {}

            def S_for(i):
                p, b = seq[i]
                pid = (p, b // 2)
                if pid not in ets:
                    ets[pid] = et_pool.tile(
                        [128, 2, 2, 512], BF, tag="et", name="et2"
                    )
                emit_S(p, b, ets[pid])

            S_for(0)
            S_for(1)
            psod = {}
            for i, (p, b) in enumerate(seq):
                if i + 2 < len(seq):
                    S_for(i + 2)
                if p < 5 and b % 2 == 0:
                    for a in qk_group_actions(*next(gsrc[p])):
                        a()
                pid = (p, b // 2)
                if b % 2 == 0:
                    psod[pid] = mm.tile([128, 512], F32, tag="mm", name="psODp")
                emit_O(p, b, ets[pid], psod[pid])
                if b % 2 == 1:
                    emit_D(p, b // 2, ets.pop(pid), psod.pop(pid))
                    if p == 5:
                        for bb in (b - 1, b):
                            for k in proj_after_b[bb]:
                                emit_proj(k)

    nc.compile()
    return nc


def _host_prep(inputs):
    x = np.asarray(inputs["x"], np.float32)
    qkv_w = np.asarray(inputs["qkv_w"], np.float32)
    q_bias = np.asarray(inputs["q_bias"], np.float32)
    v_bias = np.asarray(inputs["v_bias"], np.float32)
    rel_table = np.asarray(inputs["rel_table"], np.float32)
    proj_w = np.asarray(inputs["proj_w"], np.float32)
    proj_b = np.asarray(inputs["proj_b"], np.float32)
    rel_index = np.asarray(inputs["rel_index"], np.int64)

    wqk_t = qkv_w[: 2 * C].T.copy()
    wqk_t[:, :C] *= SCALE  # fold q scale into weights (exact: power of 2)
    # [c, j*128+col] -> [kr, j, 128*ct+col]
    wqk_np = np.ascontiguousarray(
        wqk_t.reshape(6, 128, 12, 128).transpose(1, 2, 0, 3).reshape(128, 12 * C)
    ).astype(BF16)
    wv_t = qkv_w[2 * C :].T
    wv_np = np.ascontiguousarray(
        wv_t.reshape(6, 128, C).transpose(1, 0, 2)
    ).astype(BF16)
    wp_t = proj_w.T
    wp_np = np.ascontiguousarray(
        wp_t.reshape(6, 128, C).transpose(1, 0, 2).reshape(128, 6 * C)
    ).astype(BF16)
    qb_np = np.ascontiguousarray((q_bias * SCALE).reshape(6, 128).T).astype(np.float32)
    pb = (proj_b + v_bias @ proj_w.T).astype(np.float32)
    pbb_np = np.ascontiguousarray(np.tile(pb.astype(BF16)[None, :], (128, 1)))

    # exp of rel-pos bias, transposed: ebT[h, m, n] = exp(rpb[n, m, h])
    rpb = rel_table[rel_index]              # [N, N, H]
    ebT = np.exp(np.transpose(rpb, (2, 1, 0)))  # [H, m, n]
    eb_np = np.zeros((128, 12, 512), np.float32)
    for p in range(6):
        for hj in range(2):
            h = 2 * p + hj
            eb_np[0:128, 2 * p + hj, 0:N] = ebT[h, 0:128, :]
            eb_np[0:69, 2 * p + hj, 256 : 256 + N] = ebT[h, 128:N, :]
    eb_np = np.ascontiguousarray(eb_np.reshape(128, 12 * 512)).astype(BF16)

    consts = {
        "wqk": wqk_np,
        "wv": wv_np,
        "wp": wp_np,
        "eb": eb_np,
        "qb": qb_np,
        "pbb": pbb_np,
    }
    in_maps = []
    for i in range(NCORES):
        xi = x[BL * i : BL * (i + 1)].reshape(M, C)
        xpad = np.zeros((MPAD, C), np.float32)
        xpad[:M] = xi
        xt = np.ascontiguousarray(
            xpad.T.reshape(6, 128, MPAD).transpose(1, 0, 2)
        ).astype(BF16)
        in_maps.append({"x": xt, **consts})
    return in_maps


def _run(inputs, trace=False):
    global _NC
    if _NC is None:
        _NC = _build()
    in_maps = _host_prep(inputs)
    res = run_bass_kernel_spmd(_NC, in_maps, core_ids=list(range(NCORES)), trace=trace)
    outs = [
        np.asarray(res.results[i]["out"]).astype(np.float32).reshape(BL, N, C)
        for i in range(NCORES)
    ]
    full = np.concatenate(outs, axis=0)
    return full, res


def kernel(**inputs) -> np.ndarray:
    full, _ = _run(inputs, trace=False)
    return full


# revision 40
# speedup vs baseline: 1.1859x; 1.1859x over previous
"""BEiT-style attention (B=64, N=197, C=768, H=12, rel-pos bias) on 8 TRN2 cores.

Data-parallel over batch: 8 batch items per core, no collectives.

Design notes (what made this fast — 232us baseline -> ~166us):
  - Engines execute their queues IN ORDER, so emission order IS the
    schedule, and the TRN2 PE has p-states (full 2.4 GHz only after ~3us
    of continuous execution; a stall drops it to 1.2 GHz). The whole
    kernel is one software-pipelined stream in which the tensor engine
    never head-blocks: per head-pair p, S/exp runs two b-iterations
    ahead of O/denom, and the qk GEMM of pair p+1 (proj chunks during
    p=5) is emitted block-wise between them as tensor filler that covers
    the exp->mul->O latency.
  - Rel-pos bias enters as a multiplicative exp(bias) table applied to
    exp(S) on the DVE (2-byte 2x mode) instead of an identity-matmul
    PSUM prefill; softmax denominators use ones-matmuls batched over
    b-pairs, and the ~5x-faster reciprocal_approx_fast custom DVE op.
  - All attention matmuls use K=128 (token chunks padded cross-batch;
    the exp(bias) table zeroes the pad rows so they contribute nothing).
  - Projection runs over 13 flat 128-token chunks (MPAD = 1664 = 13*128);
    proj bias is added by the DVE during PSUM evacuation (no ones-row
    matmul). v_bias is folded into the proj bias on the host (softmax
    rows sum to 1); q_bias/scale fold into the qk weights / ACT evac.
  - Every DRAM input is host-prearranged to its exact SBUF layout; the
    wv/x loads are split so the first v matmul starts ~5us earlier, and
    ~22 warmup matmuls on dummy data keep the PE busy (and its p-state
    ramped) during the input-DMA wait — the v stage then starts at full
    clock with all inputs resident.
    (Empirically, finer DMA splitting [3+ pieces of a hot tile] or
    fine-grained interleaving of accumulation groups makes ALL matmuls
    ~20% slower — keep hot tiles to <=2 DMA pieces and keep matmul
    accumulation groups contiguous.)
"""

import numpy as np
import ml_dtypes

import concourse.bass as bass
import concourse.mybir as mybir
import concourse.tile as tile
from concourse import bacc
from concourse.bass_utils import run_bass_kernel_spmd

BF16 = ml_dtypes.bfloat16
F32 = mybir.dt.float32
BF = mybir.dt.bfloat16

B, N, C = 64, 197, 768
H, HD = 12, 64
NCORES = 8
BL = B // NCORES            # 8 batches per core
M = BL * N                  # 1576 real tokens per core
MPAD = 1664                 # 13 * 128
SCALE = HD ** -0.5
MCH = [(0, 512), (512, 512), (1024, 512), (1536, 128)]
FCH = [(0, 512), (512, 256)]

_NC = None


def _build():
    nc = bacc.Bacc("TRN2", target_bir_lowering=False, debug=False)

    x = nc.dram_tensor("x", [128, 6, MPAD], BF, kind="ExternalInput")
    wqk = nc.dram_tensor("wqk", [128, 12 * C], BF, kind="ExternalInput")
    wv = nc.dram_tensor("wv", [128, 6, C], BF, kind="ExternalInput")
    wp = nc.dram_tensor("wp", [128, 6 * C], BF, kind="ExternalInput")
    eb = nc.dram_tensor("eb", [128, 12 * 512], BF, kind="ExternalInput")
    qb = nc.dram_tensor("qb", [128, 6], F32, kind="ExternalInput")
    pbb = nc.dram_tensor("pbb", [128, C], BF, kind="ExternalInput")
    out = nc.dram_tensor("out", [M, C], BF, kind="ExternalOutput")

    Ident = mybir.ActivationFunctionType.Identity
    Copy = mybir.ActivationFunctionType.Copy
    Exp = mybir.ActivationFunctionType.Exp

    with tile.TileContext(nc) as tc:
        with (
            tc.tile_pool(name="persist", bufs=1) as P,
            tc.tile_pool(name="et", bufs=5) as et_pool,
            tc.tile_pool(name="rcp", bufs=4) as r_pool,
            tc.tile_pool(name="ob", bufs=3) as ob_pool,
            tc.tile_pool(name="mm", bufs=8, space="PSUM") as mm,
        ):
            # ---- inputs to SBUF (one DMA per tensor, pre-laid-out) ----
            # wv/x split by first use; more than 2 pieces per hot tile
            # measurably slows every matmul down (see module docstring).
            wv_sb = P.tile([128, 6, C], BF, tag="wv")
            xT = P.tile([128, 6, MPAD], BF, tag="xt")
            # staged by first use: the first v matmuls (batch 0, f-half 0)
            # need only wv cols 0:512 and x tokens 0:256; the 640 split
            # point buys ~9.6us of v compute before the x tail is needed
            nc.sync.dma_start(wv_sb[:, :, 0:512], wv[:, :, 0:512])
            nc.sync.dma_start(xT[:, :, 0:640], x[:, :, 0:640])
            nc.sync.dma_start(wv_sb[:, :, 512:C], wv[:, :, 512:C])
            nc.sync.dma_start(xT[:, :, 640:MPAD], x[:, :, 640:MPAD])
            wqk_sb = P.tile([128, 12, C], BF, tag="wqk")
            nc.sync.dma_start(wqk_sb[:, :, :], wqk[:, :])
            eb_sb = P.tile([128, 12, 512], BF, tag="eb")
            nc.sync.dma_start(eb_sb[:, :, :], eb[:, :])
            wp_sb = P.tile([128, 6, C], BF, tag="wp")
            nc.sync.dma_start(wp_sb[:, :, :], wp[:, :])
            qb_sb = P.tile([128, 6], F32, tag="qb")
            nc.sync.dma_start(qb_sb[:, :], qb[:, :])
            pbb_sb = P.tile([128, C], BF, tag="pbb")
            nc.sync.dma_start(pbb_sb[:, :], pbb[:, :])

            ones64 = P.tile([128, 64], BF, tag="ones64")
            nc.gpsimd.memset(ones64[:, :], 1.0)

            # Warmup: the PE p-state ramps to full clock only after ~3us
            # of continuous execution, and the first real matmul waits
            # ~15us for input DMA. Keep the PE busy on dummy data during
            # the wait so the v stage starts at full clock (~3us saved;
            # the dummies themselves finish right as the inputs land).
            dummy = P.tile([128, 512], BF, tag="dummy")
            nc.gpsimd.memset(dummy[:, :], 0.0)
            for w in range(23):
                wps = mm.tile([128, 512], F32, tag="mm", name="warm")
                nc.tensor.matmul(
                    wps[:64, :],
                    lhsT=ones64[:, :],
                    rhs=dummy[:, :],
                    start=True,
                    stop=True,
                )

            qkT = P.tile([128, 12, MPAD], BF, tag="qkt")
            v_sb = P.tile([128, BL, 2, C], BF, tag="v")
            AT = P.tile([128, 6, MPAD], BF, tag="at")
            # zero the pad-token tail so proj chunk 12 reads finite values
            nc.gpsimd.memset(AT[:, :, M:MPAD], 0.0)

            # ---- stage V: v[m, d] per-batch 128-token chunks ----
            # ch1 rows 69:128 hold cross-batch/pad tokens; their E rows are
            # zeroed by the exp(bias) table so they never contribute.
            for b in range(BL):
                for ch in range(2):
                    t0 = b * N + 128 * ch
                    for f0, fw in FCH:
                        ps = mm.tile([128, 512], F32, tag="mm")
                        for ct in range(6):
                            nc.tensor.matmul(
                                ps[:, :fw],
                                lhsT=xT[:, ct, t0 : t0 + 128],
                                rhs=wv_sb[:, ct, f0 : f0 + fw],
                                start=(ct == 0),
                                stop=(ct == 5),
                            )
                        nc.scalar.activation(
                            v_sb[:, b, ch, f0 : f0 + fw], ps[:, :fw], Copy
                        )

            # ---- p-loop with software pipelining ----
            # Engines execute their queues IN ORDER, so the emission order
            # IS the schedule. Per pair p, S/exp/mul runs two b-iterations
            # ahead of O/denom, and the qk GEMM groups of pair p+1 (proj
            # chunks during p=5) are interleaved between them as tensor
            # filler — the tensor queue never head-blocks on the
            # exp->mul->O dependency chain and the PE stays at full
            # p-state.

            def emit_qk_group(r, p_of_r, chunks):
                for a in qk_group_actions(r, p_of_r, chunks):
                    a()

            def qk_group_actions(r, p_of_r, chunks):
                # two m-chunks per group, ct-inner with both banks held:
                # consecutive matmuls share the same stationary weights.
                # Returned as single-matmul thunks so the b-loop can pump
                # them between the short attention matmuls — their long
                # streams hide the attention stationary-load time.
                pss = []

                def mk(ct, ci, m0, mw):
                    def t():
                        if ct == 0 and ci == 0:
                            pss[:] = [
                                mm.tile([128, 512], F32, tag="mm", name=f"qkps{i}")
                                for i in range(len(chunks))
                            ]
                        nc.tensor.matmul(
                            pss[ci][:, :mw],
                            lhsT=wqk_sb[:, r, 128 * ct : 128 * (ct + 1)],
                            rhs=xT[:, ct, m0 : m0 + mw],
                            start=(ct == 0),
                            stop=(ct == 5),
                        )
                    return t

                def ev():
                    for ps, (m0, mw) in zip(pss, chunks):
                        if r == p_of_r:  # q rows: scaled bias during evac
                            nc.scalar.activation(
                                qkT[:, r, m0 : m0 + mw],
                                ps[:, :mw],
                                Ident,
                                bias=qb_sb[:, p_of_r : p_of_r + 1],
                            )
                        else:  # k rows: plain cast on the DVE
                            nc.vector.tensor_copy(
                                qkT[:, r, m0 : m0 + mw], ps[:, :mw]
                            )

                acts = [
                    mk(ct, ci, m0, mw)
                    for ct in range(6)
                    for ci, (m0, mw) in enumerate(chunks)
                ]
                acts.append(ev)
                return acts

            def qk_groups(p):
                # q tail chunk stops at M (S reads q only to 1576); k tail
                # extends to 1635 (ch1 cross-batch lhsT columns)
                for lo in (True, False):
                    for r in (p, 6 + p):
                        if lo:
                            yield (r, p, ((0, 512), (512, 512)))
                        else:
                            yield (r, p, ((1024, 512), (1536, 40 if r == p else 99)))

            def emit_S(p, b, et2):
                # S^T[m, n] per head in its own bank; the odd head's lhsT
                # sits at partition base 64 (disjoint PE row groups run
                # concurrently; drains go to different banks). Then
                # exp(S^T) -> E0 and E = E0 * exp(bias) on the DVE
                # (2-byte 2x mode); the table zeroes junk cols 197:256
                # and the ch1 pad rows 69:128. Results land in half b%2
                # of the pair tile et2.
                bn = b * N
                bi = b % 2
                psH = [
                    mm.tile([128, 512], F32, tag="mm", name=f"psH{hj}")
                    for hj in range(2)
                ]
                for hj in range(2):
                    hs = slice(64 * hj, 64 * (hj + 1))
                    for ch in range(2):
                        nc.tensor.matmul(
                            psH[hj][:, 256 * ch : 256 * ch + N],
                            lhsT=qkT[hs, 6 + p, bn + 128 * ch : bn + 128 * ch + 128],
                            rhs=qkT[hs, p, bn : bn + N],
                            start=True,
                            stop=True,
                        )
                for hj in range(2):
                    nc.scalar.activation(
                        et2[:, bi, hj, 0:453], psH[hj][:, 0:453], Exp
                    )
                nc.vector.tensor_mul(
                    et2[:, bi, :, 0:453],
                    et2[:, bi, :, 0:453],
                    eb_sb[:, 2 * p : 2 * p + 2, 0:453],
                )

            def emit_O(p, b, et2, psODp):
                # O^T for batch b into column half b%2 of the pair bank
                bi = b % 2
                for hj in range(2):
                    od = slice(64 * hj, 64 * (hj + 1))
                    for ch in range(2):
                        nc.tensor.matmul(
                            psODp[od, 256 * bi : 256 * bi + N],
                            lhsT=v_sb[:, b, ch, (2 * p + hj) * HD : (2 * p + hj + 1) * HD],
                            rhs=et2[:, bi, hj, 256 * ch : 256 * ch + N],
                            start=(ch == 0),
                            stop=(ch == 1),
                        )

            def emit_D(p, k, et2, psODp):
                # denominators for the whole b-pair: per head, one
                # accumulation over both token chunks with the pair's two
                # E tiles side by side in the free dim (2*197 cols) —
                # half the matmuls and stationary loads of per-b denoms
                psD = mm.tile([128, 512], F32, tag="mm", name="psD")
                for hj in range(2):
                    od = slice(64 * hj, 64 * (hj + 1))
                    for ch in range(2):
                        nc.tensor.matmul(
                            psD[od, 0 : 2 * N],
                            lhsT=ones64[:, :],
                            rhs=et2[:, 0:2, hj, 256 * ch : 256 * ch + N],
                            start=(ch == 0),
                            stop=(ch == 1),
                        )
                rcp = r_pool.tile([128, 2 * N], F32, tag="rcp")
                nc.vector.reciprocal_approx_fast(
                    out=rcp[:, :], in_=psD[:, 0 : 2 * N]
                )
                for bi in range(2):
                    bn = (2 * k + bi) * N
                    nc.vector.tensor_mul(
                        AT[:, p, bn : bn + N],
                        psODp[:, 256 * bi : 256 * bi + N],
                        rcp[:, N * bi : N * bi + N],
                    )

            def emit_proj(k):
                m0 = 128 * k
                nv = min(128, M - m0)  # valid rows (last chunk: 40)
                ob = ob_pool.tile([128, C], BF, tag="ob")
                for f0, fw in FCH:
                    ps = mm.tile([128, 512], F32, tag="mm")
                    for ct in range(6):
                        nc.tensor.matmul(
                            ps[:, :fw],
                            lhsT=AT[:, ct, m0 : m0 + 128],
                            rhs=wp_sb[:, ct, f0 : f0 + fw],
                            start=(ct == 0),
                            stop=(ct == 5),
                        )
                    nc.vector.tensor_add(
                        ob[:nv, f0 : f0 + fw],
                        ps[:nv, :fw],
                        pbb_sb[:nv, f0 : f0 + fw],
                    )
                nc.sync.dma_start(out[m0 : m0 + nv, :], ob[:nv, :])

            # proj chunk k is ready once attn(p=5, b) is done for all
            # batches its 128-token window touches
            proj_after_b = [[] for _ in range(BL)]
            for k in range(13):
                proj_after_b[min((128 * k + 127) // N, BL - 1)].append(k)

            for g in qk_groups(0):  # prologue: pair 0's projections
                emit_qk_group(*g)
            seq = [(p, b) for p in range(6) for b in range(BL)]
            gsrc = {p: iter(qk_groups(p + 1)) for p in range(5)}

            ets = {}

            def S_for(i):
                p, b = seq[i]
                pid = (p, b // 2)
                if pid not in ets:
                    ets[pid] = et_pool.tile(
                        [128, 2, 2, 512], BF, tag="et", name="et2"
                    )
                emit_S(p, b, ets[pid])

            S_for(0)
            S_for(1)
            psod = {}
            for i, (p, b) in enumerate(seq):
                if i + 2 < len(seq):
                    S_for(i + 2)
                if p < 5 and b % 2 == 0:
                    for a in qk_group_actions(*next(gsrc[p])):
                        a()
                pid = (p, b // 2)
                if b % 2 == 0:
                    psod[pid] = mm.tile([128, 512], F32, tag="mm", name="psODp")
                emit_O(p, b, ets[pid], psod[pid])
                if b % 2 == 1:
                    emit_D(p, b // 2, ets.pop(pid), psod.pop(pid))
                    if p == 5:
                        for bb in (b - 1, b):
                            for k in proj_after_b[bb]:
                                emit_proj(k)

    nc.compile()
    return nc


def _host_prep(inputs):
    x = np.asarray(inputs["x"], np.float32)
    qkv_w = np.asarray(inputs["qkv_w"], np.float32)
    q_bias = np.asarray(inputs["q_bias"], np.float32)
    v_bias = np.asarray(inputs["v_bias"], np.float32)
    rel_table = np.asarray(inputs["rel_table"], np.float32)
    proj_w = np.asarray(inputs["proj_w"], np.float32)
    proj_b = np.asarray(inputs["proj_b"], np.float32)
    rel_index = np.asarray(inputs["rel_index"], np.int64)

    wqk_t = qkv_w[: 2 * C].T.copy()
    wqk_t[:, :C] *= SCALE  # fold q scale into weights (exact: power of 2)
    # [c, j*128+col] -> [kr, j, 128*ct+col]
    wqk_np = np.ascontiguousarray(
        wqk_t.reshape(6, 128, 12, 128).transpose(1, 2, 0, 3).reshape(128, 12 * C)
    ).astype(BF16)
    wv_t = qkv_w[2 * C :].T
    wv_np = np.ascontiguousarray(
        wv_t.reshape(6, 128, C).transpose(1, 0, 2)
    ).astype(BF16)
    wp_t = proj_w.T
    wp_np = np.ascontiguousarray(
        wp_t.reshape(6, 128, C).transpose(1, 0, 2).reshape(128, 6 * C)
    ).astype(BF16)
    qb_np = np.ascontiguousarray((q_bias * SCALE).reshape(6, 128).T).astype(np.float32)
    pb = (proj_b + v_bias @ proj_w.T).astype(np.float32)
    pbb_np = np.ascontiguousarray(np.tile(pb.astype(BF16)[None, :], (128, 1)))

    # exp of rel-pos bias, transposed: ebT[h, m, n] = exp(rpb[n, m, h])
    rpb = rel_table[rel_index]              # [N, N, H]
    ebT = np.exp(np.transpose(rpb, (2, 1, 0)))  # [H, m, n]
    eb_np = np.zeros((128, 12, 512), np.float32)
    for p in range(6):
        for hj in range(2):
            h = 2 * p + hj
            eb_np[0:128, 2 * p + hj, 0:N] = ebT[h, 0:128, :]
            eb_np[0:69, 2 * p + hj, 256 : 256 + N] = ebT[h, 128:N, :]
    eb_np = np.ascontiguousarray(eb_np.reshape(128, 12 * 512)).astype(BF16)

    consts = {
        "wqk": wqk_np,
        "wv": wv_np,
        "wp": wp_np,
        "eb": eb_np,
        "qb": qb_np,
        "pbb": pbb_np,
    }
    in_maps = []
    for i in range(NCORES):
        xi = x[BL * i : BL * (i + 1)].reshape(M, C)
        xpad = np.zeros((MPAD, C), np.float32)
        xpad[:M] = xi
        xt = np.ascontiguousarray(
            xpad.T.reshape(6, 128, MPAD).transpose(1, 0, 2)
        ).astype(BF16)
        in_maps.append({"x": xt, **consts})
    return in_maps


def _run(inputs, trace=False):
    global _NC
    if _NC is None:
        _NC = _build()
    in_maps = _host_prep(inputs)
    res = run_bass_kernel_spmd(_NC, in_maps, core_ids=list(range(NCORES)), trace=trace)
    outs = [
        np.asarray(res.results[i]["out"]).astype(np.float32).reshape(BL, N, C)
        for i in range(NCORES)
    ]
    full = np.concatenate(outs, axis=0)
    return full, res


def kernel(**inputs) -> np.ndarray:
    full, _ = _run(inputs, trace=False)
    return full
